# revision 1
# baseline (speedup 1.0000x reference)
"""Trainium2 Bass kernel for nn_ConvDS (2x2 pixel-unshuffle + 4x4 grouped 1x1 conv).

Reference math (scale=2, H=W=1024, no padding needed):
    xr[b,c,i,hs,ws] = x[b, c, 2*hs + i//2, 2*ws + i%2]        (i = 2*dy + dx)
    out[b, j*C + c, hs, ws] = sum_i W[j,i] * xr[b,c,i,hs,ws]

Sharding: pure data parallel over batch B=16 -> 2 images per core on 8 cores.

Per-core layout trick: view each [1024, 1024] image as [512, 2048] so one
SBUF partition holds an output row's two source rows contiguously:
    free dim = [r0 (1024 interleaved a,b) | r1 (1024 interleaved c,d)]
VectorE Haar butterfly over stride-2 views (2 ops/element, the minimum for
an exact 4-point Hadamard transform), ScalarE applies the per-row scales
(0.25 for Haar), HWDGE DMAs in/out. This handles any conv_weights whose
rows are scalar multiples of Hadamard rows; a general-W fallback covers
arbitrary weights.
"""

import numpy as np

import concourse.mybir as mybir
import concourse.tile as tile
from concourse import bacc
from concourse.bass_utils import run_bass_kernel_spmd

N_CORES = 8
B, C, H, W = 16, 3, 1024, 1024
Hs, Ws = H // 2, W // 2  # 512, 512
BP = B // N_CORES  # batches per core
F32 = mybir.dt.float32

TILE_P = 128  # partitions (output rows hs) per block
BLK_F = 2 * W  # free dim per block: two image rows per partition
N_BLOCKS = Hs // TILE_P  # 4 row-blocks per image

# Hadamard sign rows in i = 2*dy + dx ordering (matches reference butterfly)
_HROWS = np.array(
    [
        [1.0, 1.0, 1.0, 1.0],
        [1.0, -1.0, 1.0, -1.0],
        [1.0, 1.0, -1.0, -1.0],
        [1.0, -1.0, -1.0, 1.0],
    ],
    dtype=np.float64,
)


def _match_hadamard(w):
    """If every row of w is (signed scalar) * a Hadamard sign row, return
    (combo_idx per row, signed scale per row); else None."""
    combos, scales = [], []
    for j in range(4):
        row = w[j].astype(np.float64)
        mag = np.abs(row)
        if mag[0] == 0 or not np.allclose(mag, mag[0], rtol=1e-6, atol=0):
            return None
        hit = None
        for k in range(4):
            if np.allclose(row, mag[0] * _HROWS[k], rtol=1e-6, atol=0):
                hit = (k, float(mag[0]))
                break
            if np.allclose(row, -mag[0] * _HROWS[k], rtol=1e-6, atol=0):
                hit = (k, float(-mag[0]))
                break
        if hit is None:
            return None
        combos.append(hit[0])
        scales.append(hit[1])
    return combos, scales


def _general_body(nc, sp, up, op, oview, X, c, t, w):
    """General 4x4 weights fallback for one [128, 2048] block."""
    va = X[:, 0:W:2]
    vb = X[:, 1:W:2]
    vc = X[:, W : 2 * W : 2]
    vd = X[:, W + 1 : 2 * W : 2]
    O = op.tile([TILE_P, 4 * Ws], F32)
    T = sp.tile([TILE_P, 4 * Ws], F32)
    U = up.tile([TILE_P, 2 * Ws], F32)
    vs = (va, vb, vc, vd)
    for j in range(4):
        for i in range(4):
            nc.vector.tensor_scalar_mul(
                T[:, i * Ws : (i + 1) * Ws], vs[i], float(w[j, i])
            )
        nc.vector.tensor_add(U[:, 0:Ws], T[:, 0:Ws], T[:, Ws : 2 * Ws])
        nc.vector.tensor_add(
            U[:, Ws : 2 * Ws], T[:, 2 * Ws : 3 * Ws], T[:, 3 * Ws : 4 * Ws]
        )
        nc.vector.tensor_add(
            O[:, j * Ws : (j + 1) * Ws], U[:, 0:Ws], U[:, Ws : 2 * Ws]
        )
    nc.scalar.dma_start(
        oview[c, t * TILE_P : (t + 1) * TILE_P],
        O[:].rearrange("p (j w) -> p j w", j=4),
    )


def _build(w, bufs=6, fuse=1, xbufs=None, warm=0):
    """Build the per-core Bass program. w: host numpy [4,4] weights.

    fuse: how many 128-row blocks one DMA / one DVE op covers.
    xbufs: input-tile buffer count (prefetch depth); defaults to bufs.
    """
    nc = bacc.Bacc(None)
    # input viewed as [BP, C, Hs, 2*W]: partition rows are output rows hs,
    # each holding its two source image rows contiguously.
    xd = nc.dram_tensor("x", [BP, C, Hs, BLK_F], F32, kind="ExternalInput")
    od = nc.dram_tensor("out", [BP, 4 * C, Hs, Ws], F32, kind="ExternalOutput")

    had = _match_hadamard(w)
    f = fuse
    assert N_BLOCKS % f == 0

    with tile.TileContext(nc) as tc:
        with (
            tc.tile_pool(name="xp", bufs=xbufs or bufs) as xp,
            tc.tile_pool(name="sp", bufs=bufs) as sp,
            tc.tile_pool(name="up", bufs=bufs) as up,
            tc.tile_pool(name="op", bufs=bufs) as op,
        ):
            idx = 0
            for b in range(BP):
                for c in range(C):
                    # DRAM output view: [c, h, j, w] with channel = j*C + c
                    oview = od[b].rearrange("(j c2) h w -> c2 h j w", j=4)
                    for tg in range(N_BLOCKS // f):
                        X = xp.tile([TILE_P, f * BLK_F], F32)
                        src = xd[
                            b, c, tg * f * TILE_P : (tg + 1) * f * TILE_P, :
                        ].rearrange("(k p) g -> p k g", k=f)
                        # during startup, alternate the issue ring so both
                        # HWDGE rings feed the SDMA engines before out-DMAs
                        # exist to occupy the ACT ring
                        in_eng = nc.scalar if idx < warm and idx % 2 else nc.sync
                        in_eng.dma_start(
                            X[:].rearrange("p (k g) -> p k g", k=f), src
                        )
                        idx += 1
                        if had is None:
                            for k in range(f):
                                _general_body(
                                    nc, sp, up, op, oview,
                                    X[:, k * BLK_F : (k + 1) * BLK_F],
                                    c, tg * f + k, w,
                                )
                            continue

                        combos, scales = had
                        # Fused Haar butterfly over f blocks at once.
                        # evens = [a_0 c_0 a_1 c_1 ...], odds = [b_0 d_0 ...]
                        ac = X[:, 0 : f * BLK_F : 2]
                        bd = X[:, 1 : f * BLK_F : 2]
                        S = sp.tile([TILE_P, f * 4 * Ws], F32)
                        half = f * 2 * Ws
                        nc.vector.tensor_add(S[:, 0:half], ac, bd)
                        nc.vector.tensor_sub(S[:, half : 2 * half], ac, bd)
                        # S layout: (g: s/d half, k: block, h: 1/2, w)
                        Sv = S[:].rearrange(
                            "p (g k h w) -> p k g h w", g=2, k=f, h=2
                        )
                        in0 = Sv[:, :, :, 0]  # [p, k, g, w]: s1_k, d1_k
                        in1 = Sv[:, :, :, 1]  # s2_k, d2_k
                        U = up.tile([TILE_P, f * 4 * Ws], F32)
                        Uv = U[:].rearrange("p (k j w) -> p k j w", k=f, j=4)
                        nc.vector.tensor_add(Uv[:, :, 0:2], in0, in1)
                        nc.vector.tensor_sub(Uv[:, :, 2:4], in0, in1)
                        O = op.tile([TILE_P, f * 4 * Ws], F32)
                        if combos == [0, 1, 2, 3] and len(set(scales)) == 1:
                            nc.scalar.mul(O[:], U[:], scales[0])
                        else:
                            for j in range(4):
                                k = combos[j]
                                for blk in range(f):
                                    jo = (blk * 4 + j) * Ws
                                    ko = (blk * 4 + k) * Ws
                                    nc.scalar.mul(
                                        O[:, jo : jo + Ws],
                                        U[:, ko : ko + Ws],
                                        scales[j],
                                    )
                        # DMA out per block: SBUF [p, (j w)] -> DRAM [h, j, w]
                        for blk in range(f):
                            t = tg * f + blk
                            nc.scalar.dma_start(
                                oview[c, t * TILE_P : (t + 1) * TILE_P],
                                O[:, blk * 4 * Ws : (blk + 1) * 4 * Ws]
                                .rearrange("p (j w) -> p j w", j=4),
                            )
    nc.compile()
    return nc


_CACHE = {}


def _get_program(w):
    key = w.tobytes()
    if key not in _CACHE:
        _CACHE[key] = _build(w)
    return _CACHE[key]


def _run(x, conv_weights, **spmd_kwargs):
    x = np.ascontiguousarray(np.asarray(x, dtype=np.float32))
    w = np.asarray(conv_weights, dtype=np.float32)
    assert x.shape == (B, C, H, W), x.shape
    nc = _get_program(w)
    in_maps = [
        {"x": x[k * BP : (k + 1) * BP].reshape(BP, C, Hs, BLK_F)}
        for k in range(N_CORES)
    ]
    res = run_bass_kernel_spmd(nc, in_maps, list(range(N_CORES)), **spmd_kwargs)
    out = np.concatenate([res.results[k]["out"] for k in range(N_CORES)], axis=0)
    return out.astype(np.float32, copy=False), res


def kernel(x, conv_weights):
    out, _ = _run(x, conv_weights)
    return out


def kernel_timed(x, conv_weights, **spmd_kwargs):
    """Run with NTFF profiling; returns (out, BassKernelResults)."""
    return _run(x, conv_weights, trace=True, **spmd_kwargs)



# revision 2
# speedup vs baseline: 1.0773x; 1.0773x over previous
"""Trainium2 Bass kernel for nn_ConvDS (2x2 pixel-unshuffle + 4x4 grouped 1x1 conv).

Reference math (scale=2, H=W=1024, no padding needed):
    xr[b,c,i,hs,ws] = x[b, c, 2*hs + i//2, 2*ws + i%2]        (i = 2*dy + dx)
    out[b, j*C + c, hs, ws] = sum_i W[j,i] * xr[b,c,i,hs,ws]

Sharding: pure data parallel over batch B=16 -> 2 images per core on 8 cores.

Fast path (exact scaled-Hadamard weights, i.e. the Haar case):
  - view each [1024, 1024] image as [512, 2048]: one SBUF partition holds an
    output row's two source rows contiguously.
  - ScalarE ACTIVATE does deinterleave (stride-2 gather) + x0.25 scale + cast
    to fp16 in one pass, producing [a|b|c|d] phase-separated fp16 data.
  - VectorE does the 4-op Hadamard butterfly on unit-stride fp16 views, which
    enables the DVE 2x packed perf mode (all operands 2-byte, step 1).
  - Output is written as fp16 (tolerance is 2e-2; fp16 error ~1e-3), halving
    output HBM traffic: per-core bytes drop 50.3MB -> 37.7MB.
  - HWDGE DMAs: input on the SP ring, output on the ACT ring.

General 4x4 weights fall back to an fp32 exact path.
"""

import numpy as np

import concourse.mybir as mybir
import concourse.tile as tile
from concourse import bacc
from concourse.bass_utils import run_bass_kernel_spmd

N_CORES = 8
B, C, H, W = 16, 3, 1024, 1024
Hs, Ws = H // 2, W // 2  # 512, 512
BP = B // N_CORES  # batches per core
F32 = mybir.dt.float32
F16 = mybir.dt.float16

TILE_P = 128  # partitions (output rows hs) per block
BLK_F = 2 * W  # free dim per block: two image rows per partition
N_BLOCKS = Hs // TILE_P  # 4 row-blocks per image

# Hadamard sign rows in i = 2*dy + dx ordering (matches reference butterfly)
_HROWS = np.array(
    [
        [1.0, 1.0, 1.0, 1.0],
        [1.0, -1.0, 1.0, -1.0],
        [1.0, 1.0, -1.0, -1.0],
        [1.0, -1.0, -1.0, 1.0],
    ],
    dtype=np.float64,
)


def _match_haar(w):
    """Return the uniform positive scale s if w == s * _HROWS (identity row
    order), else None."""
    w = np.asarray(w, dtype=np.float64)
    s = w[0, 0]
    if s <= 0:
        return None
    if np.allclose(w, s * _HROWS, rtol=1e-6, atol=0):
        return float(s)
    return None


# ---------------------------------------------------------------------------
# Fast fp16 Haar path
# ---------------------------------------------------------------------------

DEFAULT_CFG = dict(
    xbufs=6,   # input f32 tile prefetch depth (8KB/partition each)
    bufs=6,    # fp16 intermediate tile buffers (4KB/partition each)
    in_eng="sync",     # engine issuing input DMAs
    out_eng="scalar",  # engine issuing output DMAs
    deint_split=1,     # 1: one ACT op per block; 2: split h into two ACT ops
)


def _build_haar_fp16(scale, cfg=None):
    """Fast path program. scale: the uniform Hadamard row scale (0.25)."""
    cfg = {**DEFAULT_CFG, **(cfg or {})}
    nc = bacc.Bacc(None)
    xd = nc.dram_tensor("x", [BP, C, Hs, BLK_F], F32, kind="ExternalInput")
    od = nc.dram_tensor("out", [BP, 4 * C, Hs, Ws], F16, kind="ExternalOutput")

    def eng(name):
        return {"sync": nc.sync, "scalar": nc.scalar, "vector": nc.vector,
                "gpsimd": nc.gpsimd}[name]

    with tile.TileContext(nc) as tc:
        with (
            tc.tile_pool(name="xp", bufs=cfg["xbufs"]) as xp,
            tc.tile_pool(name="yp", bufs=cfg["bufs"]) as yp,
            tc.tile_pool(name="tp", bufs=cfg["bufs"]) as tp,
            tc.tile_pool(name="op", bufs=cfg["bufs"]) as op,
        ):
            for b in range(BP):
                # DRAM output view: [c, h, j, w] with channel = j*C + c
                oview = od[b].rearrange("(j c2) h w -> c2 h j w", j=4)
                for c in range(C):
                    for t in range(N_BLOCKS):
                        X = xp.tile([TILE_P, BLK_F], F32)
                        eng(cfg["in_eng"]).dma_start(
                            X[:], xd[b, c, t * TILE_P : (t + 1) * TILE_P, :]
                        )
                        # deinterleave + scale + cast -> Y = [a|b|c|d] fp16
                        # source element (h, w, par) at offset h*1024 + 2w + par
                        # dest   element (h, par, w) at offset h*1024 + par*512 + w
                        Y = yp.tile([TILE_P, BLK_F], F16)
                        if cfg["deint_split"] == 1:
                            src = X[:].rearrange(
                                "p (h w par) -> p h par w", h=2, w=Ws, par=2
                            )
                            dst = Y[:].rearrange(
                                "p (h par w) -> p h par w", h=2, par=2
                            )
                            nc.scalar.mul(dst, src, scale)
                        else:
                            for h in range(2):
                                src = X[:, h * W : (h + 1) * W].rearrange(
                                    "p (w par) -> p par w", par=2
                                )
                                dst = Y[:, h * W : (h + 1) * W].rearrange(
                                    "p (par w) -> p par w", par=2
                                )
                                nc.scalar.mul(dst, src, scale)
                        # stage 1: horizontal butterfly (unit-stride fp16)
                        # T = [pq = A+B | q... ] layout: [A+B | C+D | A-B | C-D]
                        T = tp.tile([TILE_P, BLK_F], F16)
                        Y4 = Y[:].rearrange("p (x w) -> p x w", x=4)
                        T4 = T[:].rearrange("p (x w) -> p x w", x=4)
                        nc.vector.tensor_add(T4[:, 0:2], Y4[:, 0::2], Y4[:, 1::2])
                        nc.vector.tensor_sub(T4[:, 2:4], Y4[:, 0::2], Y4[:, 1::2])
                        # stage 2: vertical butterfly -> O = [o0|o1|o2|o3]
                        O = op.tile([TILE_P, BLK_F], F16)
                        O4 = O[:].rearrange("p (x w) -> p x w", x=4)
                        nc.vector.tensor_add(O4[:, 0:2], T4[:, 0::2], T4[:, 1::2])
                        nc.vector.tensor_sub(O4[:, 2:4], T4[:, 0::2], T4[:, 1::2])
                        # out: SBUF [p, (j w)] -> DRAM [h, j, w]
                        eng(cfg["out_eng"]).dma_start(
                            oview[c, t * TILE_P : (t + 1) * TILE_P],
                            O[:].rearrange("p (j w) -> p j w", j=4),
                        )
    nc.compile()
    return nc


# ---------------------------------------------------------------------------
# General fp32 fallback (arbitrary 4x4 weights) — exact
# ---------------------------------------------------------------------------

def _build_general(w, bufs=6):
    nc = bacc.Bacc(None)
    xd = nc.dram_tensor("x", [BP, C, Hs, BLK_F], F32, kind="ExternalInput")
    od = nc.dram_tensor("out", [BP, 4 * C, Hs, Ws], F32, kind="ExternalOutput")

    with tile.TileContext(nc) as tc:
        with (
            tc.tile_pool(name="xp", bufs=bufs) as xp,
            tc.tile_pool(name="sp", bufs=bufs) as sp,
            tc.tile_pool(name="up", bufs=bufs) as up,
            tc.tile_pool(name="op", bufs=bufs) as op,
        ):
            for b in range(BP):
                for c in range(C):
                    oview = od[b].rearrange("(j c2) h w -> c2 h j w", j=4)
                    for t in range(N_BLOCKS):
                        X = xp.tile([TILE_P, BLK_F], F32)
                        nc.sync.dma_start(
                            X[:], xd[b, c, t * TILE_P : (t + 1) * TILE_P, :]
                        )
                        va = X[:, 0:W:2]
                        vb = X[:, 1:W:2]
                        vc = X[:, W : 2 * W : 2]
                        vd = X[:, W + 1 : 2 * W : 2]
                        O = op.tile([TILE_P, 4 * Ws], F32)
                        T = sp.tile([TILE_P, 4 * Ws], F32)
                        U = up.tile([TILE_P, 2 * Ws], F32)
                        vs = (va, vb, vc, vd)
                        for j in range(4):
                            for i in range(4):
                                nc.vector.tensor_scalar_mul(
                                    T[:, i * Ws : (i + 1) * Ws], vs[i], float(w[j, i])
                                )
                            nc.vector.tensor_add(
                                U[:, 0:Ws], T[:, 0:Ws], T[:, Ws : 2 * Ws]
                            )
                            nc.vector.tensor_add(
                                U[:, Ws : 2 * Ws],
                                T[:, 2 * Ws : 3 * Ws],
                                T[:, 3 * Ws : 4 * Ws],
                            )
                            nc.vector.tensor_add(
                                O[:, j * Ws : (j + 1) * Ws],
                                U[:, 0:Ws],
                                U[:, Ws : 2 * Ws],
                            )
                        nc.scalar.dma_start(
                            oview[c, t * TILE_P : (t + 1) * TILE_P],
                            O[:].rearrange("p (j w) -> p j w", j=4),
                        )
    nc.compile()
    return nc


_CACHE = {}


def _get_program(w, cfg=None):
    key = (w.tobytes(), repr(sorted((cfg or {}).items())))
    if key not in _CACHE:
        s = _match_haar(w)
        if s is not None:
            _CACHE[key] = _build_haar_fp16(s, cfg)
        else:
            _CACHE[key] = _build_general(w)
    return _CACHE[key]


def _run(x, conv_weights, cfg=None, **spmd_kwargs):
    x = np.ascontiguousarray(np.asarray(x, dtype=np.float32))
    w = np.asarray(conv_weights, dtype=np.float32)
    assert x.shape == (B, C, H, W), x.shape
    nc = _get_program(w, cfg)
    in_maps = [
        {"x": x[k * BP : (k + 1) * BP].reshape(BP, C, Hs, BLK_F)}
        for k in range(N_CORES)
    ]
    res = run_bass_kernel_spmd(nc, in_maps, list(range(N_CORES)), **spmd_kwargs)
    out = np.concatenate([res.results[k]["out"] for k in range(N_CORES)], axis=0)
    return out.astype(np.float32, copy=False), res


def kernel(x, conv_weights):
    out, _ = _run(x, conv_weights)
    return out


def kernel_timed(x, conv_weights, **spmd_kwargs):
    """Run with NTFF profiling; returns (out, BassKernelResults)."""
    return _run(x, conv_weights, trace=True, **spmd_kwargs)


# revision 15
# speedup vs baseline: 1.2736x; 1.1822x over previous
"""Trainium2 Bass kernel for nn_ConvDS (2x2 pixel-unshuffle + 4x4 grouped 1x1 conv).

Reference math (scale=2, H=W=1024, no padding needed):
    xr[b,c,i,hs,ws] = x[b, c, 2*hs + i//2, 2*ws + i%2]        (i = 2*dy + dx)
    out[b, j*C + c, hs, ws] = sum_i W[j,i] * xr[b,c,i,hs,ws]

Sharding: pure data parallel over batch B=16 -> 2 images per core on 8 cores.

Fast path (exact scaled-Hadamard weights, i.e. the Haar case):
  - view each [1024, 1024] image as [512, 2048]: one SBUF partition holds an
    output row's two source rows contiguously.
  - ScalarE ACTIVATE does deinterleave (stride-2 gather) + x0.25 scale + cast
    to fp16 in one pass, producing [a|b|c|d] phase-separated fp16 data.
  - VectorE does the 4-op Hadamard butterfly on unit-stride fp16 views, which
    enables the DVE 2x packed perf mode (all operands 2-byte, step 1).
  - Output is written as fp16 (tolerance is 2e-2; fp16 error ~1e-3), halving
    output HBM traffic: per-core bytes drop 50.3MB -> 37.7MB.
  - HWDGE DMAs: input on the SP ring, output on the ACT ring.

General 4x4 weights fall back to an fp32 exact path.
"""

import numpy as np

import concourse.mybir as mybir
import concourse.tile as tile
from concourse import bacc
from concourse.bass_utils import run_bass_kernel_spmd

N_CORES = 8
B, C, H, W = 16, 3, 1024, 1024
Hs, Ws = H // 2, W // 2  # 512, 512
BP = B // N_CORES  # batches per core
F32 = mybir.dt.float32
F16 = mybir.dt.float16

TILE_P = 128  # partitions (output rows hs) per block
BLK_F = 2 * W  # free dim per block: two image rows per partition
N_BLOCKS = Hs // TILE_P  # 4 row-blocks per image

# Hadamard sign rows in i = 2*dy + dx ordering (matches reference butterfly)
_HROWS = np.array(
    [
        [1.0, 1.0, 1.0, 1.0],
        [1.0, -1.0, 1.0, -1.0],
        [1.0, 1.0, -1.0, -1.0],
        [1.0, -1.0, -1.0, 1.0],
    ],
    dtype=np.float64,
)


def _match_haar(w):
    """Return the uniform positive scale s if w == s * _HROWS (identity row
    order), else None."""
    w = np.asarray(w, dtype=np.float64)
    s = w[0, 0]
    if s <= 0:
        return None
    if np.allclose(w, s * _HROWS, rtol=1e-6, atol=0):
        return float(s)
    return None


# ---------------------------------------------------------------------------
# Fast fp16 Haar path
# ---------------------------------------------------------------------------

DEFAULT_CFG = dict(
    xbufs=8,   # input f32 tile prefetch depth (8KB/partition each)
    bufs=7,    # fp16 intermediate tile buffers (4KB/partition each)
    in_eng="sync",     # engine(s) issuing input DMAs (str or tuple cycled)
    out_eng="gpsimd",  # SWDGE out: concatenates the 4x1KB j-runs into ~3.6KB
                       # packets and keeps issue off ScalarE/Sync
    deint_eng="scalar",  # deint+scale+cast rides ScalarE's free ACT affine
    tail_split=4,      # split the final block into column sub-blocks to
                       # shorten the drain-out chain at the end
    out_last_eng="scalar",  # last out-DMAs on HWDGE so GpSimd's expensive
                            # SWDGE drain overlaps the final block
    out_dtype="f16",   # "f16", or "i8": quantized int8 DRAM output, cast in
                       # the SWDGE out-DMA; host decodes by multiplying OUT_Q
                       # (measured slower than f16 — cast path defeats packet
                       # concatenation; kept for reference)
)

# int8 output quantization step: covers |out| up to 127*OUT_Q = 2.8 with the
# reference max |out| ~2.45, while the q/2 rounding error (~0.011 abs) stays
# well inside the 2e-2 relative-max tolerance.
OUT_Q = 2.8 / 127.0


def _build_haar_fp16(scale, cfg=None):
    """Fast path program. scale: the uniform Hadamard row scale (0.25)."""
    cfg = {**DEFAULT_CFG, **(cfg or {})}
    nc = bacc.Bacc(None)
    int8_out = cfg["out_dtype"] == "i8"
    odt = mybir.dt.int8 if int8_out else F16
    act_scale = scale / OUT_Q if int8_out else scale
    xd = nc.dram_tensor("x", [BP, C, Hs, BLK_F], F32, kind="ExternalInput")
    od = nc.dram_tensor("out", [BP, 4 * C, Hs, Ws], odt, kind="ExternalOutput")
    nc._out_quant = OUT_Q if int8_out else None

    def eng(spec, idx):
        if isinstance(spec, (tuple, list)):
            spec = spec[idx % len(spec)]
        return {"sync": nc.sync, "scalar": nc.scalar, "vector": nc.vector,
                "gpsimd": nc.gpsimd}[spec]

    with tile.TileContext(nc) as tc:
        with (
            tc.tile_pool(name="xp", bufs=cfg["xbufs"]) as xp,
            tc.tile_pool(name="yp", bufs=cfg["bufs"]) as yp,
            tc.tile_pool(name="tp", bufs=cfg["bufs"]) as tp,
            tc.tile_pool(name="op", bufs=cfg["bufs"]) as op,
        ):
            idx = 0

            def emit(b, c, t, w0, w1, idx, out_over=None):
                """One pipeline unit: rows [t*128,(t+1)*128), out cols [w0,w1)."""
                oview = od[b].rearrange("(j c2) h w -> c2 h j w", j=4)
                rows = slice(t * TILE_P, (t + 1) * TILE_P)
                ws = w1 - w0
                X = xp.tile([TILE_P, 2 * 2 * ws], F32)
                if ws == Ws:
                    eng(cfg["in_eng"], idx).dma_start(X[:], xd[b, c, rows, :])
                else:
                    src_d = xd[b, c, rows, :].rearrange("p (h e) -> p h e", h=2)
                    eng(cfg["in_eng"], idx).dma_start(
                        X[:].rearrange("p (h e) -> p h e", h=2),
                        src_d[:, :, 2 * w0 : 2 * w1],
                    )
                # deinterleave + scale + cast -> Y = [a|b|c|d] fp16
                # source element (h, w, par) at offset h*2ws + 2w + par
                # dest   element (h, par, w) at offset h*2ws + par*ws + w
                Y = yp.tile([TILE_P, 4 * ws], F16)
                src = X[:].rearrange("p (h w par) -> p h par w", h=2, w=ws, par=2)
                dst = Y[:].rearrange("p (h par w) -> p h par w", h=2, par=2)
                de = eng(cfg["deint_eng"], idx)
                if de is nc.scalar:
                    nc.scalar.mul(dst, src, act_scale)
                else:
                    de.tensor_scalar_mul(dst, src, act_scale)
                # stage 1: horizontal butterfly (unit-stride fp16)
                # T layout: [A+B | C+D | A-B | C-D]
                T = tp.tile([TILE_P, 4 * ws], F16)
                Y4 = Y[:].rearrange("p (x w) -> p x w", x=4)
                T4 = T[:].rearrange("p (x w) -> p x w", x=4)
                nc.vector.tensor_add(T4[:, 0:2], Y4[:, 0::2], Y4[:, 1::2])
                nc.vector.tensor_sub(T4[:, 2:4], Y4[:, 0::2], Y4[:, 1::2])
                # stage 2: vertical butterfly -> O = [o0|o1|o2|o3]
                O = op.tile([TILE_P, 4 * ws], F16)
                O4 = O[:].rearrange("p (x w) -> p x w", x=4)
                nc.vector.tensor_add(O4[:, 0:2], T4[:, 0::2], T4[:, 1::2])
                nc.vector.tensor_sub(O4[:, 2:4], T4[:, 0::2], T4[:, 1::2])
                # out: SBUF [p, (j w)] -> DRAM [h, j, w]
                eng(out_over or cfg["out_eng"], idx).dma_start(
                    oview[c, rows, :, w0:w1],
                    O[:].rearrange("p (j w) -> p j w", j=4),
                )

            units = [(b, c, t) for b in range(BP) for c in range(C)
                     for t in range(N_BLOCKS)]
            ts = cfg["tail_split"]
            for u, (b, c, t) in enumerate(units):
                last = u == len(units) - 1
                oo = cfg["out_last_eng"] if last else None
                if last and ts > 1:
                    step = Ws // ts
                    for k in range(ts):
                        emit(b, c, t, k * step, (k + 1) * step, idx, out_over=oo)
                        idx += 1
                else:
                    emit(b, c, t, 0, Ws, idx, out_over=oo)
                    idx += 1
    nc.compile()
    return nc


# ---------------------------------------------------------------------------
# General fp32 fallback (arbitrary 4x4 weights) — exact
# ---------------------------------------------------------------------------

def _build_general(w, bufs=6):
    nc = bacc.Bacc(None)
    xd = nc.dram_tensor("x", [BP, C, Hs, BLK_F], F32, kind="ExternalInput")
    od = nc.dram_tensor("out", [BP, 4 * C, Hs, Ws], F32, kind="ExternalOutput")

    with tile.TileContext(nc) as tc:
        with (
            tc.tile_pool(name="xp", bufs=bufs) as xp,
            tc.tile_pool(name="sp", bufs=bufs) as sp,
            tc.tile_pool(name="up", bufs=bufs) as up,
            tc.tile_pool(name="op", bufs=bufs) as op,
        ):
            for b in range(BP):
                for c in range(C):
                    oview = od[b].rearrange("(j c2) h w -> c2 h j w", j=4)
                    for t in range(N_BLOCKS):
                        X = xp.tile([TILE_P, BLK_F], F32)
                        nc.sync.dma_start(
                            X[:], xd[b, c, t * TILE_P : (t + 1) * TILE_P, :]
                        )
                        va = X[:, 0:W:2]
                        vb = X[:, 1:W:2]
                        vc = X[:, W : 2 * W : 2]
                        vd = X[:, W + 1 : 2 * W : 2]
                        O = op.tile([TILE_P, 4 * Ws], F32)
                        T = sp.tile([TILE_P, 4 * Ws], F32)
                        U = up.tile([TILE_P, 2 * Ws], F32)
                        vs = (va, vb, vc, vd)
                        for j in range(4):
                            for i in range(4):
                                nc.vector.tensor_scalar_mul(
                                    T[:, i * Ws : (i + 1) * Ws], vs[i], float(w[j, i])
                                )
                            nc.vector.tensor_add(
                                U[:, 0:Ws], T[:, 0:Ws], T[:, Ws : 2 * Ws]
                            )
                            nc.vector.tensor_add(
                                U[:, Ws : 2 * Ws],
                                T[:, 2 * Ws : 3 * Ws],
                                T[:, 3 * Ws : 4 * Ws],
                            )
                            nc.vector.tensor_add(
                                O[:, j * Ws : (j + 1) * Ws],
                                U[:, 0:Ws],
                                U[:, Ws : 2 * Ws],
                            )
                        nc.scalar.dma_start(
                            oview[c, t * TILE_P : (t + 1) * TILE_P],
                            O[:].rearrange("p (j w) -> p j w", j=4),
                        )
    nc.compile()
    return nc


_CACHE = {}


def _get_program(w, cfg=None):
    key = (w.tobytes(), repr(sorted((cfg or {}).items())))
    if key not in _CACHE:
        s = _match_haar(w)
        if s is not None:
            _CACHE[key] = _build_haar_fp16(s, cfg)
        else:
            _CACHE[key] = _build_general(w)
    return _CACHE[key]


def _run(x, conv_weights, cfg=None, **spmd_kwargs):
    x = np.ascontiguousarray(np.asarray(x, dtype=np.float32))
    w = np.asarray(conv_weights, dtype=np.float32)
    assert x.shape == (B, C, H, W), x.shape
    nc = _get_program(w, cfg)
    in_maps = [
        {"x": x[k * BP : (k + 1) * BP].reshape(BP, C, Hs, BLK_F)}
        for k in range(N_CORES)
    ]
    res = run_bass_kernel_spmd(nc, in_maps, list(range(N_CORES)), **spmd_kwargs)
    out = np.concatenate([res.results[k]["out"] for k in range(N_CORES)], axis=0)
    q = getattr(nc, "_out_quant", None)
    if q is not None:
        out = out.astype(np.float32) * np.float32(q)
    return out.astype(np.float32, copy=False), res


def kernel(x, conv_weights):
    out, _ = _run(x, conv_weights)
    return out


def kernel_timed(x, conv_weights, **spmd_kwargs):
    """Run with NTFF profiling; returns (out, BassKernelResults)."""
    return _run(x, conv_weights, trace=True, **spmd_kwargs)
